# revision 10
# baseline (speedup 1.0000x reference)
"""Causal multi-head self-attention (B=4, T=2048, D=1024, H=16) on 8 TRN2 cores.

Sharding: core c = b*2 + hh handles batch b (of 4) and head-half hh (of 2):
8 heads = 512 of the 1024 W_q/W_k/W_v output rows (column-parallel) and the
matching 512 W_o input columns (row-parallel). Each core returns its 8 heads'
k/v and a partial out projection; the host sums the two partials per batch.

Per-core dataflow (layouts chosen so no on-device transposes of big tensors
are needed; weights and x arrive pre-transposed from the host):
  phase 1: QT,KT = (Wq/Wk slice)@x^T in (d x T) layout, V in (T x d) layout
           with a ones-column appended (V_aug) for softmax denominators.
  phase 2 (per q-chunk of 512, per head): scoresT = K^T-tiles.T @ QT (keys on
           partitions), exp via ACT (fused *1/sqrt(hd)), causal diag mask
           multiply, AV with lhsT=exp-tile, rhs=V_aug -> (q x 65) PSUM where
           col 64 accumulates the softmax denominator; normalize via
           reciprocal + per-partition tensor_scalar -> y (T x 512).
  phase 3 (interleaved per q-chunk): PE-transpose y -> yT,
           out_partial = yT.T @ Wo-slice^T.

Matmul dtypes: float32r (TF32-like, 11 mantissa bits, full PE rate at
free>=256) for projections / QK / out-proj; AV in fp16 (the exp weights are
in (0, e^~6] so fp16's 11-bit mantissa + fp32 PSUM accumulate keeps the
end-to-end error at the f32r level while running 4x faster than fp32 AV).
"""

import sys

sys.path.insert(0, "/opt/trn_rl_repo")

import numpy as np

import concourse.bass as bass
import concourse.mybir as mybir
import concourse.tile as tile
from concourse import bacc
from concourse.bass_utils import run_bass_kernel_spmd
from concourse.masks import make_identity

P = 128
B, T, D = 4, 2048, 1024
H, HD = 16, 64
N_CORES = 8
DC = 512            # head dims per core (8 heads)
HPC = 8             # heads per core
T_TILES = T // P    # 16
QC = 4              # q-chunks of 512
MT = DC // P        # 4 m-tiles of head dims
DM_KT = D // P      # 8 k-tiles over d_model

F32 = mybir.dt.float32
F32R = mybir.dt.float32r
F16 = mybir.dt.float16
EXP = mybir.ActivationFunctionType.Exp


def _round_f32r(x: np.ndarray) -> np.ndarray:
    """Round f32 to the PE's float32r format (11 mantissa bits, round-half-up)."""
    b = np.ascontiguousarray(np.asarray(x, np.float32)).view(np.uint32)
    return ((b + np.uint32(1 << 11)) & np.uint32(~np.uint32((1 << 12) - 1))).view(
        np.float32
    )


def build_kernel():
    nc = bacc.Bacc("TRN2", target_bir_lowering=False, debug=False,
                   num_devices=N_CORES)

    x_t = nc.dram_tensor("x_t", [D, T], F32R, kind="ExternalInput")
    wq_t = nc.dram_tensor("wq_t", [D, DC], F32R, kind="ExternalInput")
    wk_t = nc.dram_tensor("wk_t", [D, DC], F32R, kind="ExternalInput")
    wv_t = nc.dram_tensor("wv_t", [D, DC], F32R, kind="ExternalInput")
    wo_t = nc.dram_tensor("wo_t", [DC, D], F32R, kind="ExternalInput")
    kt_o = nc.dram_tensor("kt_o", [DC, T], F32, kind="ExternalOutput")
    v_o = nc.dram_tensor("v_o", [HPC, T, HD], F32, kind="ExternalOutput")
    out_p = nc.dram_tensor("out_p", [T, D], F32, kind="ExternalOutput")

    x_t3 = x_t.rearrange("(kt p) t -> kt p t", p=P)
    wq3 = wq_t.rearrange("(kt p) m -> kt p m", p=P)
    wk3 = wk_t.rearrange("(kt p) m -> kt p m", p=P)
    wv3 = wv_t.rearrange("(kt p) m -> kt p m", p=P)
    wo3 = wo_t.rearrange("(kt p) n -> kt p n", p=P)

    with tile.TileContext(nc) as tc:
        with (
            tc.tile_pool(name="outer", bufs=1) as outer,
            tc.tile_pool(name="ps_main", bufs=2, space="PSUM") as ps_main,
            tc.tile_pool(name="ps_qk", bufs=2, space="PSUM") as ps_qk,
            tc.tile_pool(name="ps_av", bufs=1, space="PSUM") as ps_av,
        ):
            ident = outer.tile([P, P], F32, tag="ident")
            make_identity(nc, ident[:])
            # diag_mask[k, q] = 1.0 if k <= q else 0.0  (f16 for DVE 4x mode)
            diag_mask = outer.tile([P, P], F16, tag="diag_mask")
            nc.gpsimd.memset(diag_mask[:], 1.0)
            nc.gpsimd.affine_select(
                out=diag_mask[:], in_=diag_mask[:],
                compare_op=mybir.AluOpType.is_ge, fill=0.0, base=0,
                pattern=[[1, P]], channel_multiplier=-1,
            )
            # y (T x DC): [q-partition, t-tile, head*64+d]
            y_all = outer.tile([P, T_TILES, DC], F32, tag="y_all")

            with tc.tile_pool(name="mid", bufs=1) as mid:
                qt_r = mid.tile([P, MT, T], F32R, tag="qt_r")
                kt_r = mid.tile([P, MT, T], F32R, tag="kt_r")
                v_aug = mid.tile([P, T_TILES, HPC, HD + 1], F16, tag="v_aug")
                nc.gpsimd.memset(v_aug[:, :, :, HD:], 1.0)

                # ---------------- phase 1: projections ----------------
                with (
                    tc.tile_pool(name="ph1", bufs=2) as ph1,
                    tc.tile_pool(name="ph1x", bufs=1) as ph1x,
                ):
                    for half in range(2):
                        t0 = half * (T // 2)
                        xt = ph1x.tile([P, DM_KT, T // 2], F32R, tag="xt")
                        for kk in range(DM_KT):
                            nc.sync.dma_start(
                                xt[:, kk, :], x_t3[kk, :, t0:t0 + T // 2]
                            )
                        # Q and K projections: out tiles (128 dq x 512 t)
                        for name, w3, dst in (("q", wq3, qt_r), ("k", wk3, kt_r)):
                            for m in range(MT):
                                wtile = ph1.tile([P, DM_KT, P], F32R,
                                                 tag=f"w_{name}")
                                for kk in range(DM_KT):
                                    nc.sync.dma_start(
                                        wtile[:, kk, :],
                                        w3[kk, :, m * P:(m + 1) * P],
                                    )
                                for tch in range(2):
                                    tc0 = t0 + tch * 512
                                    ps = ps_main.tile([P, 512], F32, tag="mm")
                                    for kk in range(DM_KT):
                                        nc.tensor.matmul(
                                            ps[:],
                                            wtile[:, kk, :],
                                            xt[:, kk,
                                               tch * 512:(tch + 1) * 512],
                                            start=(kk == 0),
                                            stop=(kk == DM_KT - 1),
                                        )
                                    nc.vector.tensor_copy(
                                        dst[:, m, tc0:tc0 + 512], ps[:]
                                    )
                                    if name == "k":
                                        stag = ph1.tile([P, 512], F32,
                                                        tag="kstag")
                                        nc.vector.tensor_copy(stag[:], ps[:])
                                        nc.sync.dma_start(
                                            kt_o[m * P:(m + 1) * P,
                                                 tc0:tc0 + 512],
                                            stag[:],
                                        )
                        # V projection: out tiles (128 t x 512 dv)
                        wvtile = ph1x.tile([P, DM_KT, DC], F32R, tag="w_v")
                        for kk in range(DM_KT):
                            nc.sync.dma_start(wvtile[:, kk, :], wv3[kk])
                        for tb in range(T // 2 // P):
                            tb_g = half * (T // 2 // P) + tb
                            ps = ps_main.tile([P, 512], F32, tag="mm")
                            for kk in range(DM_KT):
                                nc.tensor.matmul(
                                    ps[:],
                                    xt[:, kk, tb * P:(tb + 1) * P],
                                    wvtile[:, kk, :],
                                    start=(kk == 0),
                                    stop=(kk == DM_KT - 1),
                                )
                            nc.vector.tensor_copy(
                                v_aug[:, tb_g, :, :HD],
                                ps[:].rearrange("p (h d) -> p h d", h=HPC),
                            )
                            vstag = ph1.tile([P, 512], F32, tag="vstag")
                            nc.vector.tensor_copy(vstag[:], ps[:])
                            for h in range(HPC):
                                nc.sync.dma_start(
                                    v_o[h, tb_g * P:(tb_g + 1) * P, :],
                                    vstag[:, h * HD:(h + 1) * HD],
                                )

                # ------- phases 2+3: attention, transpose, out-proj -------
                with tc.tile_pool(name="ph23", bufs=4) as ph23:
                    with tc.tile_pool(name="ph23y", bufs=1) as ph23y:
                        yt = ph23y.tile([P, MT, T], F32R, tag="yt")
                        wo_sb = ph23y.tile([P, MT, D], F32R, tag="wo_sb")
                        for hs in range(MT):
                            nc.sync.dma_start(wo_sb[:, hs, :], wo3[hs])

                        for qc in range(QC):
                            q0 = qc * 512
                            n_kt = 4 * qc + 4
                            for h in range(HPC):
                                mt, pb = h // 2, (h % 2) * 64
                                psav = [
                                    ps_av.tile([P, HD + 1], F32,
                                               tag=f"av{i}", name=f"av{i}")
                                    for i in range(4)
                                ]
                                for kt in range(n_kt):
                                    j = kt - 4 * qc
                                    ql = max(0, 128 * j)
                                    ps = ps_qk.tile([P, 512], F32, tag="qk")
                                    nc.tensor.matmul(
                                        ps[:, ql:],
                                        kt_r[pb:pb + 64, mt,
                                             kt * P:(kt + 1) * P],
                                        qt_r[pb:pb + 64, mt,
                                             q0 + ql:q0 + 512],
                                        start=True, stop=True,
                                    )
                                    et = ph23.tile([P, 512], F16, tag="expT")
                                    nc.scalar.activation(
                                        et[:, ql:], ps[:, ql:], EXP,
                                        scale=0.125,
                                    )
                                    if j >= 0:
                                        nc.vector.tensor_mul(
                                            out=et[:, ql:ql + P],
                                            in0=et[:, ql:ql + P],
                                            in1=diag_mask[:],
                                        )
                                    for i in range(max(j, 0), 4):
                                        nc.tensor.matmul(
                                            psav[i][:],
                                            et[:, i * P:(i + 1) * P],
                                            v_aug[:, kt, h, :],
                                            start=(kt == 0),
                                            stop=(kt == 4 * qc + i),
                                        )
                                for i in range(4):
                                    rec = ph23.tile([P, 1], F32, tag="rec")
                                    nc.vector.reciprocal(
                                        rec[:], psav[i][:, HD:]
                                    )
                                    nc.vector.tensor_scalar_mul(
                                        y_all[:, 4 * qc + i,
                                              h * HD:(h + 1) * HD],
                                        psav[i][:, :HD],
                                        rec[:],
                                    )
                            # transpose + out-projection for this q-chunk
                            for i in range(4):
                                qt = 4 * qc + i
                                for hs in range(MT):
                                    pst = ps_qk.tile([P, 512], F32, tag="qk")
                                    nc.tensor.transpose(
                                        pst[:, :P],
                                        y_all[:, qt, hs * P:(hs + 1) * P],
                                        ident[:],
                                    )
                                    nc.vector.tensor_copy(
                                        yt[:, hs, qt * P:(qt + 1) * P],
                                        pst[:, :P],
                                    )
                                for nh in range(2):
                                    ps = ps_main.tile([P, 512], F32, tag="mm")
                                    for hs in range(MT):
                                        nc.tensor.matmul(
                                            ps[:],
                                            yt[:, hs, qt * P:(qt + 1) * P],
                                            wo_sb[:, hs,
                                                  nh * 512:(nh + 1) * 512],
                                            start=(hs == 0),
                                            stop=(hs == MT - 1),
                                        )
                                    stag = ph23.tile([P, 512], F32,
                                                     tag="ostag")
                                    nc.vector.tensor_copy(stag[:], ps[:])
                                    nc.sync.dma_start(
                                        out_p[qt * P:(qt + 1) * P,
                                              nh * 512:(nh + 1) * 512],
                                        stag[:],
                                    )

    nc.compile()
    return nc


_NC_CACHE = None


def _get_nc():
    global _NC_CACHE
    if _NC_CACHE is None:
        _NC_CACHE = build_kernel()
    return _NC_CACHE


def kernel(x, W_q, W_k, W_v, W_o):
    x = np.asarray(x, np.float32)
    W_q = np.asarray(W_q, np.float32)
    W_k = np.asarray(W_k, np.float32)
    W_v = np.asarray(W_v, np.float32)
    W_o = np.asarray(W_o, np.float32)

    nc = _get_nc()
    wo_full_t = np.ascontiguousarray(W_o.T)
    in_maps = []
    for c in range(N_CORES):
        b, hh = divmod(c, 2)
        sl = slice(hh * DC, (hh + 1) * DC)
        in_maps.append({
            "x_t": _round_f32r(np.ascontiguousarray(x[b].T)),
            "wq_t": _round_f32r(np.ascontiguousarray(W_q[sl, :].T)),
            "wk_t": _round_f32r(np.ascontiguousarray(W_k[sl, :].T)),
            "wv_t": _round_f32r(np.ascontiguousarray(W_v[sl, :].T)),
            "wo_t": _round_f32r(np.ascontiguousarray(wo_full_t[sl, :])),
        })
    res = run_bass_kernel_spmd(nc, in_maps, core_ids=list(range(N_CORES)))

    out = np.empty((B, T, D), np.float32)
    k = np.empty((B, H, T, HD), np.float32)
    v = np.empty((B, H, T, HD), np.float32)
    for c in range(N_CORES):
        b, hh = divmod(c, 2)
        r = res.results[c]
        for h in range(HPC):
            k[b, hh * HPC + h] = r["kt_o"][h * HD:(h + 1) * HD, :].T
            v[b, hh * HPC + h] = r["v_o"][h]
    for b in range(B):
        out[b] = (
            res.results[2 * b]["out_p"].astype(np.float64)
            + res.results[2 * b + 1]["out_p"].astype(np.float64)
        ).astype(np.float32)
    return out, k, v
